# revision 39
# baseline (speedup 1.0000x reference)
"""Trainium2 Bass kernel for nn_Decoder (3-stage point-cloud decoder:
gather-upsample + concat-skip + 1x1conv (+BN+LeakyReLU) x2, final plain conv).

v2 strategy (8-core SPMD), restructured from the 522us baseline:
  - All three skip loads and their weight products (u3=s3@W1b, u2=s2@W2b,
    u1=s1@W3b) have no cross-stage dependencies: they stream from t=0,
    fully overlapped.  The serial chain only flows through tiny z-tables:
    z1 -> gather -> y1 -> BN-AR -> z2 -> gather -> y2 -> BN-AR -> z3 -> out.
  - u matmuls run channel-major (moving dim = points) in float32r
    (1 cyc/row vs 4 for fp32; operands typed f32r end-to-end so the BIR
    verifier sees f32r producers).  z3T/u1/out are bf16 (tolerance 2e-2).
  - Gathers use dma_gather(prepare_only=True): descriptor generation runs
    during phase A (needs indices only); trigger_dma after the z-table
    store puts just the data movement on the critical path.  z tables are
    stored partition-major so the z store is one contiguous DMA.
  - Lrelu -> Prelu so the whole ACT function set fits one table
    (sqrt_and_others): no 1.3us ACT_TABLE_LOAD swaps mid-kernel.
  - y-assembly fuses add + BN sum via scalar_tensor_tensor accum_out
    (in place: y += transposed gather psum); sumsq via ACT Square accum.
  - u1 = skip1@W3b + b3 is staged through DRAM in bf16 (SBUF is too small
    to keep it resident) and prefetched back during the AR2 window.
  - Stage-3: class-grouped broadcast adds u1 + bcast(z3T), split
    DVE/GpSimd; out stored as bf16.
  - Points of each stage are sharded by the core that owns their gather
    source, so every gather is core-local; only two BN-stats AllReduces.
  - BN pad columns are exactly zero on the y tensors, so global stats
    divide by the true N.
"""

import sys

sys.path.insert(0, "/opt/trn_rl_repo")

import numpy as np

from concourse import bacc, bass, bass_utils, masks, mybir, tile

dt = mybir.dt
AF = mybir.ActivationFunctionType
OP = mybir.AluOpType
AX = mybir.AxisListType

NCORES = 8
EPS = 1e-5
SLOPE = 0.01

N3, N2, N1, N0 = 4096, 16384, 65536, 262144
C3 = 512  # coarse bucket size = N3 // NCORES

GMAX = 1024  # indices per dma_gather chunk (65 ring descs each)
PIECE = 2048  # max out-add piece columns
SLAB = 4096  # out store slab columns
U1_SPLIT = 64  # all s1 chunks emitted before the y2/AR2 block


def _ceil_to(x, m):
    return ((x + m - 1) // m) * m


def _gplan(p0, p1):
    out = []
    off = p0
    while off < p1:
        ln = min(GMAX, p1 - off)
        out.append((off, ln))
        off += ln
    return out


def _wrap_idx(idx, plan):
    """[n] int -> [128, n//16] int16, wrapped per plan block, replicated
    across 16-partition groups (dma_gather idx layout)."""
    n = len(idx)
    out = np.empty((128, n // 16), np.int16)
    for off, ln in plan:
        w = idx[off : off + ln].reshape(ln // 16, 16).T.astype(np.int16)
        out[:, off // 16 : (off + ln) // 16] = np.tile(w, (8, 1))
    return out


def _bucket(owner, ncores):
    """owner: [n] core id per element -> (perm lists, positions, counts)."""
    order = np.argsort(owner, kind="stable")
    counts = np.bincount(owner, minlength=ncores)
    splits = np.split(order, np.cumsum(counts)[:-1])
    pos = np.empty(len(owner), np.int64)
    for c in range(ncores):
        pos[splits[c]] = np.arange(counts[c])
    return splits, pos, counts


def _perm_pm(q, T):
    """logical row q -> physical row in a partition-major [128, T]-block
    DRAM table (partition p holds rows p*T..p*T+T-1 contiguously)."""
    return (q % 128) * T + q // 128


# ---------------------------------------------------------------------------
# device program
# ---------------------------------------------------------------------------

PHASES = ["pre", "z1", "u3", "y1", "ar1", "u2", "z2", "u1a", "y2", "ar2",
          "u1b", "z3", "out"]


def _build_program(n1p, n2p, n3p, slabs, stop_after=None, no_prep=False):
    """slabs: tuple of (slab0, slen, pieces) where pieces is a tuple of
    (k, g0, s0, ck): out cols [g0, g0+ck) = z3T[:, s0 : s0 + ck//k] each
    broadcast k times, plus u1 cols [g0, g0+ck)."""
    lim = PHASES.index(stop_after) if stop_after else len(PHASES) - 1

    def on(ph):
        return PHASES.index(ph) <= lim

    T1 = n1p // 128
    T2 = n2p // 128
    C1 = n1p // 512  # 512-col psum chunks
    C2 = n2p // 512
    CU1 = n3p // 1024  # s1 load chunks

    nc = bacc.Bacc(
        "TRN2",
        target_bir_lowering=False,
        debug=False,
        num_devices=NCORES,
        num_swdge_queues=4,
        dynamic_dma_scratch_size=16384,
    )

    f32 = dt.float32
    bf16 = dt.bfloat16
    i16 = dt.int16
    f32r = dt.float32r

    def mmr(ps, lhsT, rhs, start, stop):
        nc.tensor.matmul(ps, lhsT, rhs, start=start, stop=stop)

    import os as _os
    _act_dma = not _os.environ.get("K_NO_ACT_DMA")

    def DMA2(dst, src_):
        (nc.scalar if _act_dma else nc.sync).dma_start(dst, src_)

    # ---- I/O ----
    featsT_h = nc.dram_tensor("featsT", [258, C3], f32, kind="ExternalInput")
    s3T_h = nc.dram_tensor("s3T", [512, n1p], f32r, kind="ExternalInput")
    s2T_h = nc.dram_tensor("s2T", [256, n2p], f32r, kind="ExternalInput")
    s1T_h = nc.dram_tensor("s1T", [128, n3p], f32r, kind="ExternalInput")
    gi1_h = nc.dram_tensor("gi1", [128, n1p // 16], i16, kind="ExternalInput")
    gi2_h = nc.dram_tensor("gi2", [128, n2p // 16], i16, kind="ExternalInput")
    W1a_h = nc.dram_tensor("W1a", [258, 192], f32, kind="ExternalInput")
    W1b_h = nc.dram_tensor("W1b", [512, 129], f32r, kind="ExternalInput")
    W2a_h = nc.dram_tensor("W2a", [129, 64], f32, kind="ExternalInput")
    W2b_h = nc.dram_tensor("W2b", [256, 64], f32r, kind="ExternalInput")
    W3a_h = nc.dram_tensor("W3a", [64, 64], bf16, kind="ExternalInput")
    W3b_h = nc.dram_tensor("W3b", [128, 34], f32r, kind="ExternalInput")
    bn1_h = nc.dram_tensor("bn1", [129, 2], f32, kind="ExternalInput")  # g, be
    bn2_h = nc.dram_tensor("bn2", [64, 2], f32, kind="ExternalInput")
    b3_h = nc.dram_tensor("b3", [34, 1], f32, kind="ExternalInput")
    out_h = nc.dram_tensor("out", [34, n3p], bf16, kind="ExternalOutput")

    def bn_scalars(sb, stats, gbe, n_true, P, name):
        """stats [P,2]=(sum,sumsq) -> s,t tiles [P,1]: s=g*rsqrt(var+eps),
        t=be-mean*s."""
        mean = sb.tile([P, 1], f32, tag=f"{name}_mean")
        ms = sb.tile([P, 1], f32, tag=f"{name}_ms")
        nc.vector.tensor_scalar(mean[:], stats[:, 0:1], 1.0 / n_true, None, OP.mult)
        nc.vector.tensor_scalar(ms[:], stats[:, 1:2], 1.0 / n_true, None, OP.mult)
        var = sb.tile([P, 1], f32, tag=f"{name}_var")
        nc.vector.tensor_tensor(var[:], mean[:], mean[:], OP.mult)
        nc.vector.tensor_tensor(var[:], ms[:], var[:], OP.subtract)
        nc.vector.tensor_scalar(var[:], var[:], EPS, None, OP.add)
        std = sb.tile([P, 1], f32, tag=f"{name}_std")
        nc.scalar.activation(std[:], var[:], AF.Sqrt)
        s = sb.tile([P, 1], f32, tag=f"{name}_s")
        nc.vector.reciprocal(s[:], std[:])
        nc.vector.tensor_tensor(s[:], s[:], gbe[:, 0:1], OP.mult)
        t = sb.tile([P, 1], f32, tag=f"{name}_t")
        nc.vector.tensor_tensor(t[:], mean[:], s[:], OP.mult)
        nc.vector.tensor_tensor(t[:], gbe[:, 1:2], t[:], OP.subtract)
        return s, t

    with tile.TileContext(nc) as tc:
        from contextlib import ExitStack

        octx = ExitStack()
        with octx:
            sb = octx.enter_context(tc.tile_pool(name="persist", bufs=1))
            dram = octx.enter_context(tc.tile_pool(name="dram", bufs=1, space="DRAM"))
            u1ps = octx.enter_context(tc.tile_pool(name="u1ps", bufs=4, space="PSUM"))

            ident = sb.tile([128, 128], f32)
            masks.make_identity(nc, ident[:])
            zrow = sb.tile([1, 192], f32)
            nc.gpsimd.memset(zrow[:], 0.0)

            # ---- small loads (SP queue first) ----
            gi1 = sb.tile([128, n1p // 16], i16)
            nc.sync.dma_start(gi1[:], gi1_h.ap())
            gi2 = sb.tile([128, n2p // 16], i16)
            nc.sync.dma_start(gi2[:], gi2_h.ap())
            W1b = sb.tile([128, 4, 129], f32r)
            for k in range(4):
                nc.sync.dma_start(W1b[:, k, :], W1b_h.ap()[k * 128 : (k + 1) * 128, :])
            W2a = sb.tile([128, 64], f32)
            W2ax = sb.tile([1, 64], f32)
            nc.sync.dma_start(W2a[:], W2a_h.ap()[0:128, :])
            nc.sync.dma_start(W2ax[:], W2a_h.ap()[128:129, :])
            W2b = sb.tile([128, 2, 64], f32r)
            for k in range(2):
                nc.sync.dma_start(W2b[:, k, :], W2b_h.ap()[k * 128 : (k + 1) * 128, :])
            W3a = sb.tile([64, 64], bf16)
            nc.sync.dma_start(W3a[:], W3a_h.ap())
            W3b = sb.tile([128, 34], f32r)
            nc.sync.dma_start(W3b[:], W3b_h.ap())
            bn1 = sb.tile([128, 2], f32)
            bn1x = sb.tile([1, 2], f32)
            nc.sync.dma_start(bn1[:], bn1_h.ap()[0:128, :])
            nc.sync.dma_start(bn1x[:], bn1_h.ap()[128:129, :])
            bn2 = sb.tile([64, 2], f32)
            nc.sync.dma_start(bn2[:], bn2_h.ap())
            b3p = sb.tile([34, 1], f32)
            nc.sync.dma_start(b3p[:], b3_h.ap())

            # shared Square scratch (x-row squares reuse partition 0)
            sq = sb.tile([128, 512], f32)

            # z tables in DRAM, partition-major blocks (see _perm_pm);
            # u1 staging table in bf16
            z1d = dram.tile([513, 192], f32)
            z2d = dram.tile([n1p + 1, 64], f32)
            u1d = dram.tile([34, n3p], bf16)

            # stage pools: creation order is the pool STACK order; pools are
            # closed LIFO.  Streams (s3/s2/s1) get regions disjoint from
            # anything live at t0 so their DMAs never wait on region WAR.
            x2ctx = ExitStack()
            st2p = x2ctx.enter_context(tc.tile_pool(name="st2p", bufs=1))
            y2T = st2p.tile([64, n2p], f32)  # u2 first, then y2 in place
            sum2 = st2p.tile([64, C2], f32)
            ssq2 = st2p.tile([64, C2], f32)
            zg2 = st2p.tile([128, T2, 64], f32)

            s1ctx = ExitStack()
            s1pool = s1ctx.enter_context(tc.tile_pool(name="s1c", bufs=4))
            u1stp = s1ctx.enter_context(tc.tile_pool(name="u1st", bufs=3))

            s23ctx = ExitStack()
            s3pool = s23ctx.enter_context(tc.tile_pool(name="s3c", bufs=6))
            s2pool = s23ctx.enter_context(tc.tile_pool(name="s2c", bufs=3))

            x1ctx = ExitStack()
            st1p = x1ctx.enter_context(tc.tile_pool(name="st1p", bufs=1))
            y1T = st1p.tile([128, n1p], f32)  # becomes x1T in place at BN
            y1Tx = st1p.tile([1, n1p], f32)
            sum1 = st1p.tile([128, T1], f32)
            sum1x = st1p.tile([1, T1], f32)
            ssq1 = st1p.tile([128, C1], f32)
            ssq1x = st1p.tile([1, C1], f32)
            zg1 = st1p.tile([128, T1, 192], f32)

            # psum pool for z1 + u3 (closed after u3 so later psum pools
            # stack above st1p's lifetime cleanly)
            uactx = ExitStack()
            upool = uactx.enter_context(
                tc.tile_pool(name="uAps", bufs=2, space="PSUM")
            )

            # ---- gather descriptor prep (indices only; data later) ----
            # NOTE: the z tables are written AFTER the preps are emitted, so
            # Tile's deferred-dep machinery has no producer edge to defer --
            # the store->trigger ordering is enforced with explicit sems.
            dma_sem1 = nc.alloc_semaphore("swdge_g1")
            dma_sem2 = [nc.alloc_semaphore(f"swdge_g2_{q}") for q in range(3)]
            sem_z1 = nc.alloc_semaphore("z1_stored")
            sem_z2 = nc.alloc_semaphore("z2_stored")
            zprobe1 = sb.tile([1, 64], f32)
            zprobe2 = sb.tile([1, 64], f32)
            pl1 = _gplan(0, n1p)
            pl2 = _gplan(0, n2p)
            for off, ln in (pl1 if not no_prep else []):
                nc.gpsimd.dma_gather(
                    zg1[:, off // 128 : (off + ln) // 128, :],
                    z1d[:],
                    gi1[:, off // 16 : (off + ln) // 16],
                    ln,
                    ln,
                    192,
                    elem_step=192,
                    prepare_only=True,
                    sem=dma_sem1,
                    queue_num=3,
                )
            for qi, (off, ln) in enumerate(pl2 if not no_prep else []):
                nc.gpsimd.dma_gather(
                    zg2[:, off // 128 : (off + ln) // 128, :],
                    z2d[:],
                    gi2[:, off // 16 : (off + ln) // 16],
                    ln,
                    ln,
                    64,
                    elem_step=64,
                    prepare_only=True,
                    sem=dma_sem2[qi % 3],
                    queue_num=qi % 3,
                )

            # ---------------- z1 = featsT.T @ W1a (partition-major store) ---
            if on("z1"):
                with (
                    nc.named_scope("ph_z1"),
                    tc.tile_pool(name="fpool", bufs=1) as fpool,
                ):
                    featsT = fpool.tile([128, 2, C3], f32)
                    featsTx = fpool.tile([2, C3], f32)
                    nc.sync.dma_start(featsT[:, 0, :], featsT_h.ap()[0:128, :])
                    nc.sync.dma_start(featsT[:, 1, :], featsT_h.ap()[128:256, :])
                    nc.sync.dma_start(featsTx[:], featsT_h.ap()[256:258, :])
                    W1a = fpool.tile([128, 2, 192], f32)
                    W1ax = fpool.tile([2, 192], f32)
                    nc.sync.dma_start(W1a[:, 0, :], W1a_h.ap()[0:128, :])
                    nc.sync.dma_start(W1a[:, 1, :], W1a_h.ap()[128:256, :])
                    nc.sync.dma_start(W1ax[:], W1a_h.ap()[256:258, :])
                    z1sb = fpool.tile([128, 4, 192], f32)
                    for t in range(4):
                        psw = upool.tile([128, 512], f32, tag="u3")
                        ps = psw[:, 0:192]
                        c0 = t * 128
                        mmr(ps, featsT[:, 0, c0 : c0 + 128], W1a[:, 0, :], True, False)
                        mmr(ps, featsT[:, 1, c0 : c0 + 128], W1a[:, 1, :], False, False)
                        mmr(ps, featsTx[:, c0 : c0 + 128], W1ax[:], False, True)
                        nc.vector.tensor_copy(z1sb[:, t, :], ps)
                    nc.sync.dma_start(
                        z1d[0:512, :].rearrange("(p t) c -> p (t c)", p=128, t=4),
                        z1sb[:],
                    )
                    nc.sync.dma_start(z1d[512:513, :], zrow[:])
                    if no_prep:
                        import os as _os2
                        g1_plan = [] if _os2.environ.get("K_NO_G1") else pl1
                        for off, ln in g1_plan:
                            nc.gpsimd.dma_gather(
                                zg1[:, off // 128 : (off + ln) // 128, :],
                                z1d[:],
                                gi1[:, off // 16 : (off + ln) // 16],
                                ln, ln, 192, elem_step=192, queue_num=3,
                            )
                    else:
                        # dummy read of z1d: RAW-ordered after both stores;
                        # its completion sem gates the trigger.
                        nc.sync.dma_start(zprobe1[:], z1d[512:513, 0:64]).then_inc(
                            sem_z1, 16
                        )
                        nc.gpsimd.trigger_dma(count=None, queue_num=3)._wait_ge(
                            sem_z1, 1
                        )

            # ---------------- phase A stage-1: s3 stream + u3 -> y1 ---------
            if on("u3"):
                with nc.named_scope("ph_u3"):
                    for ch in range(C1):
                        c0 = ch * 512
                        ps = upool.tile([128, 512], f32, tag="u3")
                        psx = upool.tile([1, 512], f32, tag="u3x")
                        for k in range(4):
                            s3k = s3pool.tile([128, 512], f32r, tag="s3c")
                            nc.sync.dma_start(
                                s3k[:],
                                s3T_h.ap()[k * 128 : (k + 1) * 128, c0 : c0 + 512],
                            )
                            mmr(ps[:], W1b[:, k, 0:128], s3k[:], k == 0, k == 3)
                            mmr(psx[:], W1b[:, k, 128:129], s3k[:], k == 0, k == 3)
                        nc.scalar.activation(y1T[:, c0 : c0 + 512], ps[:], AF.Identity)
                        nc.scalar.activation(y1Tx[:, c0 : c0 + 512], psx[:], AF.Identity)
            uactx.close()

            # zg1 transposes + fused add/sum into y1T, then sumsq + stats
            if on("y1"):
                with (
                    nc.named_scope("ph_y1"),
                    tc.tile_pool(name="tp1", bufs=2, space="PSUM") as tp1,
                ):
                    for t in range(T1):
                        c0 = t * 128
                        ps = tp1.tile([128, 2, 128], f32, tag="tp")
                        nc.tensor.transpose(ps[:, 0, :], zg1[:, t, 0:128], ident[:])
                        nc.tensor.transpose(ps[0:1, 1, :], zg1[:, t, 128:129], ident[:])
                        nc.vector.scalar_tensor_tensor(
                            y1T[:, c0 : c0 + 128],
                            ps[:, 0, :],
                            1.0,
                            y1T[:, c0 : c0 + 128],
                            OP.mult,
                            OP.add,
                            accum_out=sum1[:, t : t + 1],
                        )
                        nc.vector.scalar_tensor_tensor(
                            y1Tx[:, c0 : c0 + 128],
                            ps[0:1, 1, :],
                            1.0,
                            y1Tx[:, c0 : c0 + 128],
                            OP.mult,
                            OP.add,
                            accum_out=sum1x[:, t : t + 1],
                        )
                    for ch in range(C1):
                        c0 = ch * 512
                        nc.scalar.activation(
                            sq[:], y1T[:, c0 : c0 + 512], AF.Square,
                            accum_out=ssq1[:, ch : ch + 1],
                        )
                        nc.scalar.activation(
                            sq[0:1, :], y1Tx[:, c0 : c0 + 512], AF.Square,
                            accum_out=ssq1x[:, ch : ch + 1],
                        )
                    st1m = sb.tile([128, 2], f32)
                    st1x = sb.tile([1, 2], f32)
                    nc.vector.tensor_reduce(st1m[:, 0:1], sum1[:], AX.X, OP.add)
                    nc.vector.tensor_reduce(st1m[:, 1:2], ssq1[:], AX.X, OP.add)
                    nc.vector.tensor_reduce(st1x[:, 0:1], sum1x[:], AX.X, OP.add)
                    nc.vector.tensor_reduce(st1x[:, 1:2], ssq1x[:], AX.X, OP.add)

            # ---------------- AR1 + BN1 (in place: y1T -> x1T) --------------
            if on("ar1"):
                with nc.named_scope("ph_ar1"):
                    ar_in = dram.tile([129, 2], f32, tag="ar1i")
                    ar_out = dram.tile([129, 2], f32, tag="ar1o")
                    nc.gpsimd.dma_start(ar_in[0:128, :], st1m[:])
                    nc.gpsimd.dma_start(ar_in[128:129, :], st1x[:])
                    nc.gpsimd.collective_compute(
                        "AllReduce",
                        OP.add,
                        ins=[ar_in.opt()],
                        outs=[ar_out.opt()],
                        replica_groups=[list(range(NCORES))],
                    )
                    rst_m = sb.tile([128, 2], f32)
                    rst_x = sb.tile([1, 2], f32)
                    nc.sync.dma_start(rst_m[:], ar_out[0:128, :])
                    nc.sync.dma_start(rst_x[:], ar_out[128:129, :])
                    s_m, t_m = bn_scalars(sb, rst_m, bn1, float(N2), 128, "bn1m")
                    s_x, t_x = bn_scalars(sb, rst_x, bn1x, float(N2), 1, "bn1x")
                    nc.scalar.activation(
                        y1T[:], y1T[:], AF.Prelu, bias=t_m[:], scale=s_m[:],
                        alpha=SLOPE,
                    )
                    nc.scalar.activation(
                        y1Tx[:], y1Tx[:], AF.Prelu, bias=t_x[:], scale=s_x[:],
                        alpha=SLOPE,
                    )
            x1T, x1Tx = y1T, y1Tx

            # ---------------- phase A stage-2: s2 stream + u2 (into y2T) ----
            if on("u2"):
                with nc.named_scope("ph_u2"), \
                        tc.tile_pool(name="u2ps", bufs=2, space="PSUM") as u2ps:
                    for ld in range(n2p // 1024):
                        l0 = ld * 1024
                        s2k = []
                        for k in range(2):
                            t_ = s2pool.tile([128, 1024], f32r, tag="s2c")
                            nc.sync.dma_start(
                                t_[:],
                                s2T_h.ap()[k * 128 : (k + 1) * 128, l0 : l0 + 1024],
                            )
                            s2k.append(t_)
                        for half in range(2):
                            c0 = l0 + half * 512
                            h0 = half * 512
                            ps = u2ps.tile([64, 512], f32, tag="u2")
                            for k in range(2):
                                mmr(
                                    ps[:], W2b[:, k, :], s2k[k][:, h0 : h0 + 512],
                                    k == 0, k == 1,
                                )
                            nc.vector.tensor_copy(y2T[:, c0 : c0 + 512], ps[:])

            # ---------------- z2 = W2a.T @ x1T (channel-major + transpose) --
            if on("z2"):
                with (
                    nc.named_scope("ph_z2"),
                    tc.tile_pool(name="z2ps", bufs=2, space="PSUM") as z2ps,
                    tc.tile_pool(name="z2tp", bufs=2, space="PSUM") as z2tp,
                ):
                    z2T = st1p.tile([64, n1p], f32)
                    z2p = st1p.tile([128, T1, 64], f32)
                    for ch in range(C1):
                        c0 = ch * 512
                        ps = z2ps.tile([64, 512], f32, tag="z2")
                        mmr(ps[:], W2a[:], x1T[:, c0 : c0 + 512], True, False)
                        mmr(ps[:], W2ax[:], x1Tx[:, c0 : c0 + 512], False, True)
                        nc.scalar.activation(z2T[:, c0 : c0 + 512], ps[:], AF.Identity)
                    for t in range(T1):
                        ps = z2tp.tile([128, 64], f32, tag="tp")
                        nc.tensor.transpose(
                            ps[:], z2T[:, t * 128 : (t + 1) * 128], ident[0:64, 0:64]
                        )
                        nc.vector.tensor_copy(z2p[:, t, :], ps[:])
                    nc.sync.dma_start(
                        z2d[0:n1p, :].rearrange("(p t) c -> p (t c)", p=128, t=T1),
                        z2p[:],
                    )
                    nc.sync.dma_start(z2d[n1p : n1p + 1, :], zrow[:, 0:64])
                    if no_prep:
                        for qi, (off, ln) in enumerate(pl2):
                            nc.gpsimd.dma_gather(
                                zg2[:, off // 128 : (off + ln) // 128, :],
                                z2d[:],
                                gi2[:, off // 16 : (off + ln) // 16],
                                ln, ln, 64, elem_step=64, queue_num=qi % 3,
                            )
                    else:
                        nc.sync.dma_start(zprobe2[:], z2d[n1p : n1p + 1, :]).then_inc(
                            sem_z2, 16
                        )
                        for q in range(3):
                            nc.gpsimd.trigger_dma(
                                count=None, queue_num=q
                            )._wait_ge(sem_z2, 1)
            x1ctx.close()
            s23ctx.close()

            # ---------------- phase A stage-3: s1 stream -> u1d (bf16) ------
            # Emitted in two parts so late s1 chunks don't clog the ACT/DVE
            # queues ahead of the stage-2 stats and AR2.
            def u1_chunk(ld):
                l0 = ld * 1024
                s1c = s1pool.tile([128, 1024], f32r, tag="s1c")
                nc.sync.dma_start(s1c[:], s1T_h.ap()[:, l0 : l0 + 1024])
                u1st = u1stp.tile([34, 1024], bf16, tag="u1st")
                for q in range(2):
                    h0 = q * 512
                    ps = u1ps.tile([34, 512], f32, tag="u1")
                    mmr(ps[:], W3b[:], s1c[:, h0 : h0 + 512], True, True)
                    if (ld + q) % 2 == 0:
                        nc.scalar.activation(
                            u1st[:, h0 : h0 + 512], ps[:], AF.Identity, bias=b3p[:]
                        )
                    else:
                        nc.vector.tensor_scalar(
                            u1st[:, h0 : h0 + 512], ps[:], b3p[:, 0:1], None, OP.add
                        )
                DMA2(u1d[:, l0 : l0 + 1024], u1st[:])

            if on("u1a"):
                with nc.named_scope("ph_u1a"):
                    for ld in range(min(U1_SPLIT, CU1)):
                        u1_chunk(ld)

            # ---------------- y2 assembly (gather2 data + u2, in place) -----
            if on("y2"):
                with (
                    nc.named_scope("ph_y2"),
                    tc.tile_pool(name="tp2", bufs=2, space="PSUM") as tp2,
                ):
                    for ch in range(C2):
                        ps = tp2.tile([64, 512], f32, tag="tp")
                        for j in range(4):
                            t = ch * 4 + j
                            nc.tensor.transpose(
                                ps[:, j * 128 : (j + 1) * 128], zg2[:, t, :], ident[:]
                            )
                        c0 = ch * 512
                        nc.vector.scalar_tensor_tensor(
                            y2T[:, c0 : c0 + 512],
                            ps[:],
                            1.0,
                            y2T[:, c0 : c0 + 512],
                            OP.mult,
                            OP.add,
                            accum_out=sum2[:, ch : ch + 1],
                        )
                    for ch in range(C2):
                        c0 = ch * 512
                        nc.scalar.activation(
                            sq[0:64, :], y2T[:, c0 : c0 + 512], AF.Square,
                            accum_out=ssq2[:, ch : ch + 1],
                        )
                    st2m = sb.tile([64, 2], f32)
                    nc.vector.tensor_reduce(st2m[:, 0:1], sum2[:], AX.X, OP.add)
                    nc.vector.tensor_reduce(st2m[:, 1:2], ssq2[:], AX.X, OP.add)

            # ---------------- AR2 ----------------
            if on("ar2"):
                with nc.named_scope("ph_ar2"):
                    ar2_in = dram.tile([64, 2], f32, tag="ar2i")
                    ar2_out = dram.tile([64, 2], f32, tag="ar2o")
                    nc.gpsimd.dma_start(ar2_in[:], st2m[:])
                    nc.gpsimd.collective_compute(
                        "AllReduce",
                        OP.add,
                        ins=[ar2_in.opt()],
                        outs=[ar2_out.opt()],
                        replica_groups=[list(range(NCORES))],
                    )
                    rst2 = sb.tile([64, 2], f32)
                    nc.sync.dma_start(rst2[:], ar2_out[:])
                    s2s, t2s = bn_scalars(sb, rst2, bn2, float(N1), 64, "bn2")

            # remaining s1 chunks execute during the AR2 window
            if on("u1b"):
                with nc.named_scope("ph_u1b"):
                    for ld in range(min(U1_SPLIT, CU1), CU1):
                        u1_chunk(ld)
            s1ctx.close()

            # ---------------- BN2 + z3 (+ u1 prefetch back) -----------------
            # st3 pools reuse the region freed by the s1/s23/st1 pools.
            st3ctx = ExitStack()
            st3p = st3ctx.enter_context(tc.tile_pool(name="st3p", bufs=1))
            u1cp = st3ctx.enter_context(tc.tile_pool(name="u1c", bufs=4))
            u1tiles = []
            if on("z3"):
                z3T = st3p.tile([34, n2p], bf16)
                for slab0, slen, _pieces in slabs:
                    u1c = u1cp.tile([34, SLAB], bf16, tag="u1c")
                    nc.sync.dma_start(u1c[:, :slen], u1d[:, slab0 : slab0 + slen])
                    u1tiles.append(u1c)
                with (
                    nc.named_scope("ph_z3"),
                    tc.tile_pool(name="x2c", bufs=3) as x2cp,
                    tc.tile_pool(name="z3ps", bufs=2, space="PSUM") as z3ps,
                ):
                    for ch in range(C2):
                        c0 = ch * 512
                        x2c = x2cp.tile([64, 512], bf16, tag="x2c")
                        nc.scalar.activation(
                            x2c[:], y2T[:, c0 : c0 + 512], AF.Prelu,
                            bias=t2s[:], scale=s2s[:], alpha=SLOPE,
                        )
                        ps = z3ps.tile([64, 512], f32, tag="z3")
                        nc.tensor.matmul(
                            ps[:], W3a[:], x2c[:], start=True, stop=True
                        )
                        nc.vector.tensor_copy(z3T[:, c0 : c0 + 512], ps[0:34, :])

            # ---------------- stage-3 out = u1 + bcast(z3T) -----------------
            if on("out"):
                with (
                    nc.named_scope("ph_out"),
                    tc.tile_pool(name="outp", bufs=3) as outp,
                ):
                    for si, (slab0, slen, pieces) in enumerate(slabs):
                        ot = outp.tile([34, SLAB], bf16, tag="ot")
                        u1c = u1tiles[si]
                        for k, g0, s0, ck in pieces:
                            nj = ck // k
                            o0 = g0 - slab0
                            nc.vector.tensor_tensor(
                                ot[:, o0 : o0 + ck].rearrange("p (j i) -> p j i", i=k),
                                z3T[:, s0 : s0 + nj]
                                .unsqueeze(2)
                                .broadcast_to([34, nj, k]),
                                u1c[:, o0 : o0 + ck].rearrange("p (j i) -> p j i", i=k),
                                OP.add,
                            )
                        DMA2(
                            out_h.ap()[:, slab0 : slab0 + slen], ot[:, :slen]
                        )
            st3ctx.close()
            x2ctx.close()

    nc.compile()
    return nc


# ---------------------------------------------------------------------------
# host wrapper
# ---------------------------------------------------------------------------

_CACHE = {}


def _get_program(key, *args):
    if key not in _CACHE:
        _CACHE[key] = _build_program(*args)
    return _CACHE[key]


def prepare(
    feats,
    skip1,
    skip2,
    skip3,
    idx1,
    idx2,
    idx3,
    W1,
    b1,
    g1,
    be1,
    W2,
    b2,
    g2,
    be2,
    W3,
    b3,
):
    """Host-side sharding/layout. Returns (build_key, in_maps, outmaps)."""
    import ml_dtypes

    feats = np.asarray(feats, np.float32)
    skip1 = np.asarray(skip1, np.float32)
    skip2 = np.asarray(skip2, np.float32)
    skip3 = np.asarray(skip3, np.float32)
    idx1 = np.asarray(idx1, np.int64)
    idx2 = np.asarray(idx2, np.int64)
    idx3 = np.asarray(idx3, np.int64)
    W1 = np.asarray(W1, np.float32)
    W2 = np.asarray(W2, np.float32)
    W3 = np.asarray(W3, np.float32)
    b3 = np.asarray(b3, np.float32)
    g1 = np.asarray(g1, np.float32)
    be1 = np.asarray(be1, np.float32)
    g2 = np.asarray(g2, np.float32)
    be2 = np.asarray(be2, np.float32)

    # ---- locality sharding ----
    own1 = idx1 // C3  # owner core of each stage-1 point
    P1, pos1, cnt1 = _bucket(own1, NCORES)
    own2 = own1[idx2]  # owner of each stage-2 point = owner of its source
    P2raw, _, cnt2 = _bucket(own2, NCORES)

    n1p = _ceil_to(max(int(cnt1.max()), 512), 512)
    T1 = n1p // 128

    # ---- stage-3 fanout classes over stage-2 points ----
    fan = np.bincount(idx3, minlength=N1)  # global fanout per stage-2 point
    KMAX = int(fan.max())
    cnt_ck = np.zeros((NCORES, KMAX + 1), np.int64)
    for c in range(NCORES):
        cnt_ck[c] = np.bincount(fan[P2raw[c]], minlength=KMAX + 1)
    M = cnt_ck.max(axis=0)
    # pad class 0 so n2p is a multiple of 2048
    M[0] += _ceil_to(int(M.sum()), 2048) - int(M.sum())
    Mtuple = tuple(int(x) for x in M)

    K = len(Mtuple)
    src_off = [0] * K
    out_off = [0] * K
    o = 0
    for k in range(K):
        src_off[k] = o
        o += Mtuple[k]
    n2p = o
    o = 0
    for k in range(1, K):
        out_off[k] = o
        o += Mtuple[k] * k
    n3p_real = o
    n3p = _ceil_to(n3p_real, 2048)

    # out-add pieces (<=PIECE cols, multiple of k) grouped into store slabs
    pieces_all = []
    for k in range(1, K):
        blk = Mtuple[k] * k
        if blk == 0:
            continue
        ck_max = (PIECE // k) * k
        done = 0
        while done < blk:
            ck = min(ck_max, blk - done)
            pieces_all.append((k, out_off[k] + done, src_off[k] + done // k, ck))
            done += ck
    slabs = []
    cur0, curlen, curp = 0, 0, []
    for pc in pieces_all:
        if curlen + pc[3] > SLAB:
            slabs.append((cur0, curlen, tuple(curp)))
            cur0, curlen, curp = cur0 + curlen, 0, []
        curp.append(pc)
        curlen += pc[3]
    if curlen:
        slabs.append((cur0, curlen, tuple(curp)))
    slabs = tuple(slabs)

    # stage-2 slot assignment: class-grouped per core
    slot2 = np.full(N1, -1, np.int64)  # global stage-2 point -> core slot
    P2 = []
    for c in range(NCORES):
        pts = P2raw[c]
        order = np.argsort(fan[pts], kind="stable")
        pts_sorted = pts[order]
        ks = fan[pts_sorted]
        slots = np.empty(len(pts), np.int64)
        for k in range(KMAX + 1):
            m = ks == k
            slots[m] = src_off[k] + np.arange(int(m.sum()))
        slot2[pts_sorted] = slots
        P2.append(pts_sorted)

    # stage-3 output mapping (CSR over idx3 by source)
    order3 = np.argsort(idx3, kind="stable")
    start = np.zeros(N1 + 1, np.int64)
    np.cumsum(fan, out=start[1:])

    outmaps = []  # per core: out col -> original stage-3 point (-1 pad)
    for c in range(NCORES):
        omap = np.full(n3p, -1, np.int64)
        pts = P2[c]
        ks = fan[pts]
        for k in range(1, KMAX + 1):
            srcs = pts[ks == k]  # already in slot order
            nk = len(srcs)
            if nk == 0:
                continue
            gidx = (start[srcs][:, None] + np.arange(k)[None, :]).reshape(-1)
            omap[out_off[k] : out_off[k] + nk * k] = order3[gidx]
        outmaps.append(omap)

    key = (n1p, n2p, n3p, slabs)

    # shared weights
    W1a = np.zeros((258, 192), np.float32)
    W1a[:, :129] = W1[:258]
    W1b = np.ascontiguousarray(W1[258:770])
    W2a = np.ascontiguousarray(W2[:129])
    W2b = np.ascontiguousarray(W2[129:385])
    W3a = np.zeros((64, 64), np.float32)
    W3a[:, :34] = W3[:64]
    W3a_bf = W3a.astype(ml_dtypes.bfloat16)
    W3b = np.ascontiguousarray(W3[64:192])
    bn1 = np.stack([g1, be1], 1)
    bn2 = np.stack([g2, be2], 1)
    b3p = np.ascontiguousarray(b3.reshape(34, 1))

    featsTf = np.ascontiguousarray(feats.T)  # [258, 4096]
    s3Tf = skip3.T
    s2Tf = skip2.T
    s1Tf = skip1.T

    pl1 = _gplan(0, n1p)
    pl2 = _gplan(0, n2p)

    in_maps = []
    for c in range(NCORES):
        p1, p2 = P1[c], P2[c]
        k1 = len(p1)
        omap = outmaps[c]

        s3T = np.zeros((512, n1p), np.float32)
        s3T[:, :k1] = s3Tf[:, p1]
        s2T = np.zeros((256, n2p), np.float32)
        s2T[:, slot2[p2]] = s2Tf[:, p2]
        s1T = np.zeros((128, n3p), np.float32)
        valid = omap >= 0
        s1T[:, valid] = s1Tf[:, omap[valid]]

        g1i = np.full(n1p, 512, np.int64)
        g1i[:k1] = _perm_pm(idx1[p1] - C3 * c, 4)

        g2i = np.full(n2p, n1p, np.int64)
        g2i[slot2[p2]] = _perm_pm(pos1[idx2[p2]], T1)

        in_maps.append(
            {
                "featsT": np.ascontiguousarray(featsTf[:, C3 * c : C3 * (c + 1)]),
                "s3T": s3T,
                "s2T": s2T,
                "s1T": s1T,
                "gi1": _wrap_idx(g1i, pl1),
                "gi2": _wrap_idx(g2i, pl2),
                "W1a": W1a,
                "W1b": W1b,
                "W2a": W2a,
                "W2b": W2b,
                "W3a": W3a_bf,
                "W3b": W3b,
                "bn1": bn1,
                "bn2": bn2,
                "b3": b3p,
            }
        )

    return key, in_maps, outmaps


def _install_ntff_hook():
    """The image's antenv lacks axon_hooks; shim it so trace=True can capture
    NTFF profiles through the axon .so (same path trn_boot would register)."""
    import sys as _sys
    import types

    if "antenv.axon_hooks" in _sys.modules:
        return
    mod = types.ModuleType("antenv.axon_hooks")
    holder = {}
    mod.set_axon_ntff_profile_hook = lambda h: holder.__setitem__("h", h)
    mod.get_axon_ntff_profile_hook = lambda: holder.get("h")
    _sys.modules["antenv.axon_hooks"] = mod
    try:
        from trn_agent_boot.trn_boot import _ntff_profile_via_ctypes

        h = _ntff_profile_via_ctypes("/opt/axon/libaxon_pjrt.so")
        if h is not None:
            holder["h"] = h
    except Exception:
        pass


def kernel(_want_trace=False, **inputs):
    import os

    if _want_trace:
        _install_ntff_hook()
    key, in_maps, outmaps = prepare(**inputs)
    stop_after = os.environ.get("K_STOP_AFTER") or None
    no_prep = not os.environ.get("K_PREP")
    key2 = key + (stop_after, no_prep)
    nc = _get_program(key2, *key, stop_after, no_prep)

    res = bass_utils.run_bass_kernel_spmd(
        nc, in_maps, core_ids=list(range(NCORES)), trace=_want_trace
    )

    out = np.empty((N0, 34), np.float32)
    for c in range(NCORES):
        omap = outmaps[c]
        valid = omap >= 0
        out[omap[valid]] = np.asarray(res.results[c]["out"], np.float32).T[valid]

    if _want_trace:
        kernel._last_trace = res
    return out


# revision 41
# speedup vs baseline: 1.2143x; 1.2143x over previous
"""Trainium2 Bass kernel for nn_Decoder (3-stage point-cloud decoder:
gather-upsample + concat-skip + 1x1conv (+BN+LeakyReLU) x2, final plain conv).

v2 strategy (8-core SPMD), restructured from the 522us baseline:
  - All three skip loads and their weight products (u3=s3@W1b, u2=s2@W2b,
    u1=s1@W3b) have no cross-stage dependencies: they stream from t=0,
    fully overlapped.  The serial chain only flows through tiny z-tables:
    z1 -> gather -> y1 -> BN-AR -> z2 -> gather -> y2 -> BN-AR -> z3 -> out.
  - u matmuls run channel-major (moving dim = points) in float32r
    (1 cyc/row vs 4 for fp32; operands typed f32r end-to-end so the BIR
    verifier sees f32r producers).  z3T/u1/out are bf16 (tolerance 2e-2).
  - Gathers use dma_gather(prepare_only=True): descriptor generation runs
    during phase A (needs indices only); trigger_dma after the z-table
    store puts just the data movement on the critical path.  z tables are
    stored partition-major so the z store is one contiguous DMA.
  - Lrelu -> Prelu so the whole ACT function set fits one table
    (sqrt_and_others): no 1.3us ACT_TABLE_LOAD swaps mid-kernel.
  - y-assembly fuses add + BN sum via scalar_tensor_tensor accum_out
    (in place: y += transposed gather psum); sumsq via ACT Square accum.
  - u1 = skip1@W3b + b3 is staged through DRAM in bf16 (SBUF is too small
    to keep it resident) and prefetched back during the AR2 window.
  - Stage-3: class-grouped broadcast adds u1 + bcast(z3T), split
    DVE/GpSimd; out stored as bf16.
  - Points of each stage are sharded by the core that owns their gather
    source, so every gather is core-local; only two BN-stats AllReduces.
  - BN pad columns are exactly zero on the y tensors, so global stats
    divide by the true N.
"""

import sys

sys.path.insert(0, "/opt/trn_rl_repo")

import numpy as np

from concourse import bacc, bass, bass_utils, masks, mybir, tile

dt = mybir.dt
AF = mybir.ActivationFunctionType
OP = mybir.AluOpType
AX = mybir.AxisListType

NCORES = 8
EPS = 1e-5
SLOPE = 0.01

N3, N2, N1, N0 = 4096, 16384, 65536, 262144
C3 = 512  # coarse bucket size = N3 // NCORES

GMAX = 1024  # indices per dma_gather chunk (65 ring descs each)
PIECE = 2048  # max out-add piece columns
SLAB = 4096  # out store slab columns
U1_SPLIT = 64  # all s1 chunks emitted before the y2/AR2 block


def _ceil_to(x, m):
    return ((x + m - 1) // m) * m


def _gplan(p0, p1):
    out = []
    off = p0
    while off < p1:
        ln = min(GMAX, p1 - off)
        out.append((off, ln))
        off += ln
    return out


def _wrap_idx(idx, plan):
    """[n] int -> [128, n//16] int16, wrapped per plan block, replicated
    across 16-partition groups (dma_gather idx layout)."""
    n = len(idx)
    out = np.empty((128, n // 16), np.int16)
    for off, ln in plan:
        w = idx[off : off + ln].reshape(ln // 16, 16).T.astype(np.int16)
        out[:, off // 16 : (off + ln) // 16] = np.tile(w, (8, 1))
    return out


def _bucket(owner, ncores):
    """owner: [n] core id per element -> (perm lists, positions, counts)."""
    order = np.argsort(owner, kind="stable")
    counts = np.bincount(owner, minlength=ncores)
    splits = np.split(order, np.cumsum(counts)[:-1])
    pos = np.empty(len(owner), np.int64)
    for c in range(ncores):
        pos[splits[c]] = np.arange(counts[c])
    return splits, pos, counts


def _perm_pm(q, T):
    """logical row q -> physical row in a partition-major [128, T]-block
    DRAM table (partition p holds rows p*T..p*T+T-1 contiguously)."""
    return (q % 128) * T + q // 128


# ---------------------------------------------------------------------------
# device program
# ---------------------------------------------------------------------------

PHASES = ["pre", "z1", "u3", "y1", "ar1", "u2", "z2", "u1a", "y2", "ar2",
          "u1b", "z3", "out"]


def _build_program(n1p, n2p, n3p, slabs, stop_after=None, no_prep=False):
    """slabs: tuple of (slab0, slen, pieces) where pieces is a tuple of
    (k, g0, s0, ck): out cols [g0, g0+ck) = z3T[:, s0 : s0 + ck//k] each
    broadcast k times, plus u1 cols [g0, g0+ck)."""
    lim = PHASES.index(stop_after) if stop_after else len(PHASES) - 1

    def on(ph):
        return PHASES.index(ph) <= lim

    T1 = n1p // 128
    T2 = n2p // 128
    C1 = n1p // 512  # 512-col psum chunks
    C2 = n2p // 512
    CU1 = n3p // 2048  # s1 load chunks

    nc = bacc.Bacc(
        "TRN2",
        target_bir_lowering=False,
        debug=False,
        num_devices=NCORES,
        num_swdge_queues=4,
        dynamic_dma_scratch_size=16384,
    )

    f32 = dt.float32
    bf16 = dt.bfloat16
    i16 = dt.int16
    f32r = dt.float32r

    def mmr(ps, lhsT, rhs, start, stop):
        nc.tensor.matmul(ps, lhsT, rhs, start=start, stop=stop)

    import os as _os
    _act_dma = not _os.environ.get("K_NO_ACT_DMA")

    def DMA2(dst, src_):
        (nc.scalar if _act_dma else nc.sync).dma_start(dst, src_)

    # ---- I/O ----
    featsT_h = nc.dram_tensor("featsT", [258, C3], f32, kind="ExternalInput")
    s3T_h = nc.dram_tensor("s3T", [512, n1p], f32r, kind="ExternalInput")
    s2T_h = nc.dram_tensor("s2T", [256, n2p], f32r, kind="ExternalInput")
    s1T_h = nc.dram_tensor("s1T", [128, n3p], f32r, kind="ExternalInput")
    gi1_h = nc.dram_tensor("gi1", [128, n1p // 16], i16, kind="ExternalInput")
    gi2_h = nc.dram_tensor("gi2", [128, n2p // 16], i16, kind="ExternalInput")
    W1a_h = nc.dram_tensor("W1a", [258, 192], f32, kind="ExternalInput")
    W1b_h = nc.dram_tensor("W1b", [512, 129], f32r, kind="ExternalInput")
    W2a_h = nc.dram_tensor("W2a", [129, 64], f32, kind="ExternalInput")
    W2b_h = nc.dram_tensor("W2b", [256, 64], f32r, kind="ExternalInput")
    W3a_h = nc.dram_tensor("W3a", [64, 64], bf16, kind="ExternalInput")
    W3b_h = nc.dram_tensor("W3b", [128, 34], f32r, kind="ExternalInput")
    bn1_h = nc.dram_tensor("bn1", [129, 2], f32, kind="ExternalInput")  # g, be
    bn2_h = nc.dram_tensor("bn2", [64, 2], f32, kind="ExternalInput")
    b3_h = nc.dram_tensor("b3", [34, 1], f32, kind="ExternalInput")
    out_h = nc.dram_tensor("out", [34, n3p], bf16, kind="ExternalOutput")

    def bn_scalars(sb, stats, gbe, n_true, P, name):
        """stats [P,2]=(sum,sumsq) -> s,t tiles [P,1]: s=g*rsqrt(var+eps),
        t=be-mean*s."""
        mean = sb.tile([P, 1], f32, tag=f"{name}_mean")
        ms = sb.tile([P, 1], f32, tag=f"{name}_ms")
        nc.vector.tensor_scalar(mean[:], stats[:, 0:1], 1.0 / n_true, None, OP.mult)
        nc.vector.tensor_scalar(ms[:], stats[:, 1:2], 1.0 / n_true, None, OP.mult)
        var = sb.tile([P, 1], f32, tag=f"{name}_var")
        nc.vector.tensor_tensor(var[:], mean[:], mean[:], OP.mult)
        nc.vector.tensor_tensor(var[:], ms[:], var[:], OP.subtract)
        nc.vector.tensor_scalar(var[:], var[:], EPS, None, OP.add)
        std = sb.tile([P, 1], f32, tag=f"{name}_std")
        nc.scalar.activation(std[:], var[:], AF.Sqrt)
        s = sb.tile([P, 1], f32, tag=f"{name}_s")
        nc.vector.reciprocal(s[:], std[:])
        nc.vector.tensor_tensor(s[:], s[:], gbe[:, 0:1], OP.mult)
        t = sb.tile([P, 1], f32, tag=f"{name}_t")
        nc.vector.tensor_tensor(t[:], mean[:], s[:], OP.mult)
        nc.vector.tensor_tensor(t[:], gbe[:, 1:2], t[:], OP.subtract)
        return s, t

    with tile.TileContext(nc) as tc:
        from contextlib import ExitStack

        octx = ExitStack()
        with octx:
            sb = octx.enter_context(tc.tile_pool(name="persist", bufs=1))
            dram = octx.enter_context(tc.tile_pool(name="dram", bufs=1, space="DRAM"))
            u1ps = octx.enter_context(tc.tile_pool(name="u1ps", bufs=3, space="PSUM"))

            ident = sb.tile([128, 128], f32)
            masks.make_identity(nc, ident[:])
            zrow = sb.tile([1, 192], f32)
            nc.gpsimd.memset(zrow[:], 0.0)

            # ---- small loads (SP queue first) ----
            gi1 = sb.tile([128, n1p // 16], i16)
            nc.sync.dma_start(gi1[:], gi1_h.ap())
            gi2 = sb.tile([128, n2p // 16], i16)
            nc.sync.dma_start(gi2[:], gi2_h.ap())
            W1b = sb.tile([128, 4, 129], f32r)
            for k in range(4):
                nc.sync.dma_start(W1b[:, k, :], W1b_h.ap()[k * 128 : (k + 1) * 128, :])
            W2a = sb.tile([128, 64], f32)
            W2ax = sb.tile([1, 64], f32)
            nc.sync.dma_start(W2a[:], W2a_h.ap()[0:128, :])
            nc.sync.dma_start(W2ax[:], W2a_h.ap()[128:129, :])
            W2b = sb.tile([128, 2, 64], f32r)
            for k in range(2):
                nc.sync.dma_start(W2b[:, k, :], W2b_h.ap()[k * 128 : (k + 1) * 128, :])
            W3a = sb.tile([64, 64], bf16)
            nc.sync.dma_start(W3a[:], W3a_h.ap())
            W3b = sb.tile([128, 34], f32r)
            nc.sync.dma_start(W3b[:], W3b_h.ap())
            bn1 = sb.tile([128, 2], f32)
            bn1x = sb.tile([1, 2], f32)
            nc.sync.dma_start(bn1[:], bn1_h.ap()[0:128, :])
            nc.sync.dma_start(bn1x[:], bn1_h.ap()[128:129, :])
            bn2 = sb.tile([64, 2], f32)
            nc.sync.dma_start(bn2[:], bn2_h.ap())
            b3p = sb.tile([34, 1], f32)
            nc.sync.dma_start(b3p[:], b3_h.ap())

            # shared Square scratch (x-row squares reuse partition 0)
            sq = sb.tile([128, 512], f32)

            # z tables in DRAM, partition-major blocks (see _perm_pm);
            # u1 staging table in bf16
            z1d = dram.tile([513, 192], f32)
            z2d = dram.tile([n1p + 1, 64], f32)
            u1d = dram.tile([34, n3p], bf16)

            # stage pools: creation order is the pool STACK order; pools are
            # closed LIFO.  Streams (s3/s2/s1) get regions disjoint from
            # anything live at t0 so their DMAs never wait on region WAR.
            x2ctx = ExitStack()
            st2p = x2ctx.enter_context(tc.tile_pool(name="st2p", bufs=1))
            y2T = st2p.tile([64, n2p], f32)  # u2 first, then y2 in place
            sum2 = st2p.tile([64, C2], f32)
            ssq2 = st2p.tile([64, C2], f32)
            zg2 = st2p.tile([128, T2, 64], f32)

            s1ctx = ExitStack()
            s1pool = s1ctx.enter_context(tc.tile_pool(name="s1c", bufs=3))
            u1stp = s1ctx.enter_context(tc.tile_pool(name="u1st", bufs=3))

            s23ctx = ExitStack()
            s3pool = s23ctx.enter_context(tc.tile_pool(name="s3c", bufs=6))
            s2pool = s23ctx.enter_context(tc.tile_pool(name="s2c", bufs=3))

            x1ctx = ExitStack()
            st1p = x1ctx.enter_context(tc.tile_pool(name="st1p", bufs=1))
            y1T = st1p.tile([128, n1p], f32)  # becomes x1T in place at BN
            y1Tx = st1p.tile([1, n1p], f32)
            sum1 = st1p.tile([128, T1], f32)
            sum1x = st1p.tile([1, T1], f32)
            ssq1 = st1p.tile([128, C1], f32)
            ssq1x = st1p.tile([1, C1], f32)
            zg1 = st1p.tile([128, T1, 192], f32)

            # psum pool for z1 + u3 (closed after u3 so later psum pools
            # stack above st1p's lifetime cleanly)
            uactx = ExitStack()
            upool = uactx.enter_context(
                tc.tile_pool(name="uAps", bufs=2, space="PSUM")
            )

            # ---- gather descriptor prep (indices only; data later) ----
            # NOTE: the z tables are written AFTER the preps are emitted, so
            # Tile's deferred-dep machinery has no producer edge to defer --
            # the store->trigger ordering is enforced with explicit sems.
            dma_sem1 = nc.alloc_semaphore("swdge_g1")
            dma_sem2 = [nc.alloc_semaphore(f"swdge_g2_{q}") for q in range(3)]
            sem_z1 = nc.alloc_semaphore("z1_stored")
            sem_z2 = nc.alloc_semaphore("z2_stored")
            zprobe1 = sb.tile([1, 64], f32)
            zprobe2 = sb.tile([1, 64], f32)
            pl1 = _gplan(0, n1p)
            pl2 = _gplan(0, n2p)
            for off, ln in (pl1 if not no_prep else []):
                nc.gpsimd.dma_gather(
                    zg1[:, off // 128 : (off + ln) // 128, :],
                    z1d[:],
                    gi1[:, off // 16 : (off + ln) // 16],
                    ln,
                    ln,
                    192,
                    elem_step=192,
                    prepare_only=True,
                    sem=dma_sem1,
                    queue_num=3,
                )
            for qi, (off, ln) in enumerate(pl2 if not no_prep else []):
                nc.gpsimd.dma_gather(
                    zg2[:, off // 128 : (off + ln) // 128, :],
                    z2d[:],
                    gi2[:, off // 16 : (off + ln) // 16],
                    ln,
                    ln,
                    64,
                    elem_step=64,
                    prepare_only=True,
                    sem=dma_sem2[qi % 3],
                    queue_num=qi % 3,
                )

            # ---------------- z1 = featsT.T @ W1a (partition-major store) ---
            if on("z1"):
                with (
                    nc.named_scope("ph_z1"),
                    tc.tile_pool(name="fpool", bufs=1) as fpool,
                ):
                    featsT = fpool.tile([128, 2, C3], f32)
                    featsTx = fpool.tile([2, C3], f32)
                    nc.sync.dma_start(featsT[:, 0, :], featsT_h.ap()[0:128, :])
                    nc.sync.dma_start(featsT[:, 1, :], featsT_h.ap()[128:256, :])
                    nc.sync.dma_start(featsTx[:], featsT_h.ap()[256:258, :])
                    W1a = fpool.tile([128, 2, 192], f32)
                    W1ax = fpool.tile([2, 192], f32)
                    nc.sync.dma_start(W1a[:, 0, :], W1a_h.ap()[0:128, :])
                    nc.sync.dma_start(W1a[:, 1, :], W1a_h.ap()[128:256, :])
                    nc.sync.dma_start(W1ax[:], W1a_h.ap()[256:258, :])
                    z1sb = fpool.tile([128, 4, 192], f32)
                    for t in range(4):
                        psw = upool.tile([128, 512], f32, tag="u3")
                        ps = psw[:, 0:192]
                        c0 = t * 128
                        mmr(ps, featsT[:, 0, c0 : c0 + 128], W1a[:, 0, :], True, False)
                        mmr(ps, featsT[:, 1, c0 : c0 + 128], W1a[:, 1, :], False, False)
                        mmr(ps, featsTx[:, c0 : c0 + 128], W1ax[:], False, True)
                        nc.vector.tensor_copy(z1sb[:, t, :], ps)
                    nc.sync.dma_start(
                        z1d[0:512, :].rearrange("(p t) c -> p (t c)", p=128, t=4),
                        z1sb[:],
                    )
                    nc.sync.dma_start(z1d[512:513, :], zrow[:])
                    if no_prep:
                        import os as _os2
                        g1_plan = [] if _os2.environ.get("K_NO_G1") else pl1
                        for off, ln in g1_plan:
                            nc.gpsimd.dma_gather(
                                zg1[:, off // 128 : (off + ln) // 128, :],
                                z1d[:],
                                gi1[:, off // 16 : (off + ln) // 16],
                                ln, ln, 192, elem_step=192, queue_num=3,
                            )
                    else:
                        # dummy read of z1d: RAW-ordered after both stores;
                        # its completion sem gates the trigger.
                        nc.sync.dma_start(zprobe1[:], z1d[512:513, 0:64]).then_inc(
                            sem_z1, 16
                        )
                        nc.gpsimd.trigger_dma(count=None, queue_num=3)._wait_ge(
                            sem_z1, 1
                        )

            # ---------------- phase A stage-1: s3 stream + u3 -> y1 ---------
            if on("u3"):
                with nc.named_scope("ph_u3"):
                    for ch in range(C1):
                        c0 = ch * 512
                        ps = upool.tile([128, 512], f32, tag="u3")
                        psx = upool.tile([1, 512], f32, tag="u3x")
                        for k in range(4):
                            s3k = s3pool.tile([128, 512], f32r, tag="s3c")
                            nc.sync.dma_start(
                                s3k[:],
                                s3T_h.ap()[k * 128 : (k + 1) * 128, c0 : c0 + 512],
                            )
                            mmr(ps[:], W1b[:, k, 0:128], s3k[:], k == 0, k == 3)
                            mmr(psx[:], W1b[:, k, 128:129], s3k[:], k == 0, k == 3)
                        nc.scalar.activation(y1T[:, c0 : c0 + 512], ps[:], AF.Identity)
                        nc.scalar.activation(y1Tx[:, c0 : c0 + 512], psx[:], AF.Identity)
            uactx.close()

            # zg1 transposes + fused add/sum into y1T, then sumsq + stats
            if on("y1"):
                with (
                    nc.named_scope("ph_y1"),
                    tc.tile_pool(name="tp1", bufs=2, space="PSUM") as tp1,
                ):
                    for t in range(T1):
                        c0 = t * 128
                        ps = tp1.tile([128, 2, 128], f32, tag="tp")
                        nc.tensor.transpose(ps[:, 0, :], zg1[:, t, 0:128], ident[:])
                        nc.tensor.transpose(ps[0:1, 1, :], zg1[:, t, 128:129], ident[:])
                        nc.vector.scalar_tensor_tensor(
                            y1T[:, c0 : c0 + 128],
                            ps[:, 0, :],
                            1.0,
                            y1T[:, c0 : c0 + 128],
                            OP.mult,
                            OP.add,
                            accum_out=sum1[:, t : t + 1],
                        )
                        nc.vector.scalar_tensor_tensor(
                            y1Tx[:, c0 : c0 + 128],
                            ps[0:1, 1, :],
                            1.0,
                            y1Tx[:, c0 : c0 + 128],
                            OP.mult,
                            OP.add,
                            accum_out=sum1x[:, t : t + 1],
                        )
                    for ch in range(C1):
                        c0 = ch * 512
                        nc.scalar.activation(
                            sq[:], y1T[:, c0 : c0 + 512], AF.Square,
                            accum_out=ssq1[:, ch : ch + 1],
                        )
                        nc.scalar.activation(
                            sq[0:1, :], y1Tx[:, c0 : c0 + 512], AF.Square,
                            accum_out=ssq1x[:, ch : ch + 1],
                        )
                    st1m = sb.tile([128, 2], f32)
                    st1x = sb.tile([1, 2], f32)
                    nc.vector.tensor_reduce(st1m[:, 0:1], sum1[:], AX.X, OP.add)
                    nc.vector.tensor_reduce(st1m[:, 1:2], ssq1[:], AX.X, OP.add)
                    nc.vector.tensor_reduce(st1x[:, 0:1], sum1x[:], AX.X, OP.add)
                    nc.vector.tensor_reduce(st1x[:, 1:2], ssq1x[:], AX.X, OP.add)

            # ---------------- AR1 + BN1 (in place: y1T -> x1T) --------------
            if on("ar1"):
                with nc.named_scope("ph_ar1"):
                    ar_in = dram.tile([129, 2], f32, tag="ar1i")
                    ar_out = dram.tile([129, 2], f32, tag="ar1o")
                    nc.gpsimd.dma_start(ar_in[0:128, :], st1m[:])
                    nc.gpsimd.dma_start(ar_in[128:129, :], st1x[:])
                    nc.gpsimd.collective_compute(
                        "AllReduce",
                        OP.add,
                        ins=[ar_in.opt()],
                        outs=[ar_out.opt()],
                        replica_groups=[list(range(NCORES))],
                    )
                    rst_m = sb.tile([128, 2], f32)
                    rst_x = sb.tile([1, 2], f32)
                    nc.sync.dma_start(rst_m[:], ar_out[0:128, :])
                    nc.sync.dma_start(rst_x[:], ar_out[128:129, :])
                    s_m, t_m = bn_scalars(sb, rst_m, bn1, float(N2), 128, "bn1m")
                    s_x, t_x = bn_scalars(sb, rst_x, bn1x, float(N2), 1, "bn1x")
                    nc.scalar.activation(
                        y1T[:], y1T[:], AF.Prelu, bias=t_m[:], scale=s_m[:],
                        alpha=SLOPE,
                    )
                    nc.scalar.activation(
                        y1Tx[:], y1Tx[:], AF.Prelu, bias=t_x[:], scale=s_x[:],
                        alpha=SLOPE,
                    )
            x1T, x1Tx = y1T, y1Tx

            # ---------------- phase A stage-2: s2 stream + u2 (into y2T) ----
            if on("u2"):
                with nc.named_scope("ph_u2"), \
                        tc.tile_pool(name="u2ps", bufs=2, space="PSUM") as u2ps:
                    for ld in range(n2p // 1024):
                        l0 = ld * 1024
                        s2k = []
                        for k in range(2):
                            t_ = s2pool.tile([128, 1024], f32r, tag="s2c")
                            nc.sync.dma_start(
                                t_[:],
                                s2T_h.ap()[k * 128 : (k + 1) * 128, l0 : l0 + 1024],
                            )
                            s2k.append(t_)
                        for half in range(2):
                            c0 = l0 + half * 512
                            h0 = half * 512
                            ps = u2ps.tile([64, 512], f32, tag="u2")
                            for k in range(2):
                                mmr(
                                    ps[:], W2b[:, k, :], s2k[k][:, h0 : h0 + 512],
                                    k == 0, k == 1,
                                )
                            nc.vector.tensor_copy(y2T[:, c0 : c0 + 512], ps[:])

            # ---------------- z2 = W2a.T @ x1T (channel-major + transpose) --
            if on("z2"):
                with (
                    nc.named_scope("ph_z2"),
                    tc.tile_pool(name="z2ps", bufs=2, space="PSUM") as z2ps,
                    tc.tile_pool(name="z2tp", bufs=2, space="PSUM") as z2tp,
                ):
                    z2T = st1p.tile([64, n1p], f32)
                    z2p = st1p.tile([128, T1, 64], f32)
                    for ch in range(C1):
                        c0 = ch * 512
                        ps = z2ps.tile([64, 512], f32, tag="z2")
                        mmr(ps[:], W2a[:], x1T[:, c0 : c0 + 512], True, False)
                        mmr(ps[:], W2ax[:], x1Tx[:, c0 : c0 + 512], False, True)
                        nc.scalar.activation(z2T[:, c0 : c0 + 512], ps[:], AF.Identity)
                    for t in range(T1):
                        ps = z2tp.tile([128, 64], f32, tag="tp")
                        nc.tensor.transpose(
                            ps[:], z2T[:, t * 128 : (t + 1) * 128], ident[0:64, 0:64]
                        )
                        nc.vector.tensor_copy(z2p[:, t, :], ps[:])
                    nc.sync.dma_start(
                        z2d[0:n1p, :].rearrange("(p t) c -> p (t c)", p=128, t=T1),
                        z2p[:],
                    )
                    nc.sync.dma_start(z2d[n1p : n1p + 1, :], zrow[:, 0:64])
                    if no_prep:
                        for qi, (off, ln) in enumerate(pl2):
                            nc.gpsimd.dma_gather(
                                zg2[:, off // 128 : (off + ln) // 128, :],
                                z2d[:],
                                gi2[:, off // 16 : (off + ln) // 16],
                                ln, ln, 64, elem_step=64, queue_num=qi % 3,
                            )
                    else:
                        nc.sync.dma_start(zprobe2[:], z2d[n1p : n1p + 1, :]).then_inc(
                            sem_z2, 16
                        )
                        for q in range(3):
                            nc.gpsimd.trigger_dma(
                                count=None, queue_num=q
                            )._wait_ge(sem_z2, 1)
            x1ctx.close()
            s23ctx.close()

            # ---------------- phase A stage-3: s1 stream -> u1d (bf16) ------
            # Emitted in two parts so late s1 chunks don't clog the ACT/DVE
            # queues ahead of the stage-2 stats and AR2.
            def u1_chunk(ld):
                l0 = ld * 2048
                s1c = s1pool.tile([128, 2048], f32r, tag="s1c")
                nc.sync.dma_start(s1c[:], s1T_h.ap()[:, l0 : l0 + 2048])
                u1st = u1stp.tile([34, 2048], bf16, tag="u1st")
                for q in range(4):
                    h0 = q * 512
                    ps = u1ps.tile([34, 512], f32, tag="u1")
                    mmr(ps[:], W3b[:], s1c[:, h0 : h0 + 512], True, True)
                    if q % 2 == 0:
                        nc.scalar.activation(
                            u1st[:, h0 : h0 + 512], ps[:], AF.Identity, bias=b3p[:]
                        )
                    else:
                        nc.vector.tensor_scalar(
                            u1st[:, h0 : h0 + 512], ps[:], b3p[:, 0:1], None, OP.add
                        )
                DMA2(u1d[:, l0 : l0 + 2048], u1st[:])

            if on("u1a"):
                with nc.named_scope("ph_u1a"):
                    for ld in range(min(U1_SPLIT, CU1)):
                        u1_chunk(ld)

            # ---------------- y2 assembly (gather2 data + u2, in place) -----
            if on("y2"):
                with (
                    nc.named_scope("ph_y2"),
                    tc.tile_pool(name="tp2", bufs=2, space="PSUM") as tp2,
                ):
                    for ch in range(C2):
                        ps = tp2.tile([64, 512], f32, tag="tp")
                        for j in range(4):
                            t = ch * 4 + j
                            nc.tensor.transpose(
                                ps[:, j * 128 : (j + 1) * 128], zg2[:, t, :], ident[:]
                            )
                        c0 = ch * 512
                        nc.vector.scalar_tensor_tensor(
                            y2T[:, c0 : c0 + 512],
                            ps[:],
                            1.0,
                            y2T[:, c0 : c0 + 512],
                            OP.mult,
                            OP.add,
                            accum_out=sum2[:, ch : ch + 1],
                        )
                    for ch in range(C2):
                        c0 = ch * 512
                        nc.scalar.activation(
                            sq[0:64, :], y2T[:, c0 : c0 + 512], AF.Square,
                            accum_out=ssq2[:, ch : ch + 1],
                        )
                    st2m = sb.tile([64, 2], f32)
                    nc.vector.tensor_reduce(st2m[:, 0:1], sum2[:], AX.X, OP.add)
                    nc.vector.tensor_reduce(st2m[:, 1:2], ssq2[:], AX.X, OP.add)

            # ---------------- AR2 ----------------
            if on("ar2"):
                with nc.named_scope("ph_ar2"):
                    ar2_in = dram.tile([64, 2], f32, tag="ar2i")
                    ar2_out = dram.tile([64, 2], f32, tag="ar2o")
                    nc.gpsimd.dma_start(ar2_in[:], st2m[:])
                    nc.gpsimd.collective_compute(
                        "AllReduce",
                        OP.add,
                        ins=[ar2_in.opt()],
                        outs=[ar2_out.opt()],
                        replica_groups=[list(range(NCORES))],
                    )
                    rst2 = sb.tile([64, 2], f32)
                    nc.sync.dma_start(rst2[:], ar2_out[:])
                    s2s, t2s = bn_scalars(sb, rst2, bn2, float(N1), 64, "bn2")

            # remaining s1 chunks execute during the AR2 window
            if on("u1b"):
                with nc.named_scope("ph_u1b"):
                    for ld in range(min(U1_SPLIT, CU1), CU1):
                        u1_chunk(ld)
            s1ctx.close()

            # ---------------- BN2 + z3 (+ u1 prefetch back) -----------------
            # st3 pools reuse the region freed by the s1/s23/st1 pools.
            st3ctx = ExitStack()
            st3p = st3ctx.enter_context(tc.tile_pool(name="st3p", bufs=1))
            u1cp = st3ctx.enter_context(tc.tile_pool(name="u1c", bufs=4))
            u1tiles = []
            if on("z3"):
                z3T = st3p.tile([34, n2p], bf16)
                for slab0, slen, _pieces in slabs:
                    u1c = u1cp.tile([34, SLAB], bf16, tag="u1c")
                    nc.sync.dma_start(u1c[:, :slen], u1d[:, slab0 : slab0 + slen])
                    u1tiles.append(u1c)
                with (
                    nc.named_scope("ph_z3"),
                    tc.tile_pool(name="x2c", bufs=3) as x2cp,
                    tc.tile_pool(name="z3ps", bufs=2, space="PSUM") as z3ps,
                ):
                    for ch in range(C2):
                        c0 = ch * 512
                        x2c = x2cp.tile([64, 512], bf16, tag="x2c")
                        nc.scalar.activation(
                            x2c[:], y2T[:, c0 : c0 + 512], AF.Prelu,
                            bias=t2s[:], scale=s2s[:], alpha=SLOPE,
                        )
                        ps = z3ps.tile([64, 512], f32, tag="z3")
                        nc.tensor.matmul(
                            ps[:], W3a[:], x2c[:], start=True, stop=True
                        )
                        nc.vector.tensor_copy(z3T[:, c0 : c0 + 512], ps[0:34, :])

            # ---------------- stage-3 out = u1 + bcast(z3T) -----------------
            if on("out"):
                with (
                    nc.named_scope("ph_out"),
                    tc.tile_pool(name="outp", bufs=3) as outp,
                ):
                    for si, (slab0, slen, pieces) in enumerate(slabs):
                        ot = outp.tile([34, SLAB], bf16, tag="ot")
                        u1c = u1tiles[si]
                        for k, g0, s0, ck in pieces:
                            nj = ck // k
                            o0 = g0 - slab0
                            nc.vector.tensor_tensor(
                                ot[:, o0 : o0 + ck].rearrange("p (j i) -> p j i", i=k),
                                z3T[:, s0 : s0 + nj]
                                .unsqueeze(2)
                                .broadcast_to([34, nj, k]),
                                u1c[:, o0 : o0 + ck].rearrange("p (j i) -> p j i", i=k),
                                OP.add,
                            )
                        DMA2(
                            out_h.ap()[:, slab0 : slab0 + slen], ot[:, :slen]
                        )
            st3ctx.close()
            x2ctx.close()

    nc.compile()
    return nc


# ---------------------------------------------------------------------------
# host wrapper
# ---------------------------------------------------------------------------

_CACHE = {}


def _get_program(key, *args):
    if key not in _CACHE:
        _CACHE[key] = _build_program(*args)
    return _CACHE[key]


def prepare(
    feats,
    skip1,
    skip2,
    skip3,
    idx1,
    idx2,
    idx3,
    W1,
    b1,
    g1,
    be1,
    W2,
    b2,
    g2,
    be2,
    W3,
    b3,
):
    """Host-side sharding/layout. Returns (build_key, in_maps, outmaps)."""
    import ml_dtypes

    feats = np.asarray(feats, np.float32)
    skip1 = np.asarray(skip1, np.float32)
    skip2 = np.asarray(skip2, np.float32)
    skip3 = np.asarray(skip3, np.float32)
    idx1 = np.asarray(idx1, np.int64)
    idx2 = np.asarray(idx2, np.int64)
    idx3 = np.asarray(idx3, np.int64)
    W1 = np.asarray(W1, np.float32)
    W2 = np.asarray(W2, np.float32)
    W3 = np.asarray(W3, np.float32)
    b3 = np.asarray(b3, np.float32)
    g1 = np.asarray(g1, np.float32)
    be1 = np.asarray(be1, np.float32)
    g2 = np.asarray(g2, np.float32)
    be2 = np.asarray(be2, np.float32)

    # ---- locality sharding ----
    own1 = idx1 // C3  # owner core of each stage-1 point
    P1, pos1, cnt1 = _bucket(own1, NCORES)
    own2 = own1[idx2]  # owner of each stage-2 point = owner of its source
    P2raw, _, cnt2 = _bucket(own2, NCORES)

    n1p = _ceil_to(max(int(cnt1.max()), 512), 512)
    T1 = n1p // 128

    # ---- stage-3 fanout classes over stage-2 points ----
    fan = np.bincount(idx3, minlength=N1)  # global fanout per stage-2 point
    KMAX = int(fan.max())
    cnt_ck = np.zeros((NCORES, KMAX + 1), np.int64)
    for c in range(NCORES):
        cnt_ck[c] = np.bincount(fan[P2raw[c]], minlength=KMAX + 1)
    M = cnt_ck.max(axis=0)
    # pad class 0 so n2p is a multiple of 2048
    M[0] += _ceil_to(int(M.sum()), 2048) - int(M.sum())
    Mtuple = tuple(int(x) for x in M)

    K = len(Mtuple)
    src_off = [0] * K
    out_off = [0] * K
    o = 0
    for k in range(K):
        src_off[k] = o
        o += Mtuple[k]
    n2p = o
    o = 0
    for k in range(1, K):
        out_off[k] = o
        o += Mtuple[k] * k
    n3p_real = o
    n3p = _ceil_to(n3p_real, 2048)

    # out-add pieces (<=PIECE cols, multiple of k) grouped into store slabs
    pieces_all = []
    for k in range(1, K):
        blk = Mtuple[k] * k
        if blk == 0:
            continue
        ck_max = (PIECE // k) * k
        done = 0
        while done < blk:
            ck = min(ck_max, blk - done)
            pieces_all.append((k, out_off[k] + done, src_off[k] + done // k, ck))
            done += ck
    slabs = []
    cur0, curlen, curp = 0, 0, []
    for pc in pieces_all:
        if curlen + pc[3] > SLAB:
            slabs.append((cur0, curlen, tuple(curp)))
            cur0, curlen, curp = cur0 + curlen, 0, []
        curp.append(pc)
        curlen += pc[3]
    if curlen:
        slabs.append((cur0, curlen, tuple(curp)))
    slabs = tuple(slabs)

    # stage-2 slot assignment: class-grouped per core
    slot2 = np.full(N1, -1, np.int64)  # global stage-2 point -> core slot
    P2 = []
    for c in range(NCORES):
        pts = P2raw[c]
        order = np.argsort(fan[pts], kind="stable")
        pts_sorted = pts[order]
        ks = fan[pts_sorted]
        slots = np.empty(len(pts), np.int64)
        for k in range(KMAX + 1):
            m = ks == k
            slots[m] = src_off[k] + np.arange(int(m.sum()))
        slot2[pts_sorted] = slots
        P2.append(pts_sorted)

    # stage-3 output mapping (CSR over idx3 by source)
    order3 = np.argsort(idx3, kind="stable")
    start = np.zeros(N1 + 1, np.int64)
    np.cumsum(fan, out=start[1:])

    outmaps = []  # per core: out col -> original stage-3 point (-1 pad)
    for c in range(NCORES):
        omap = np.full(n3p, -1, np.int64)
        pts = P2[c]
        ks = fan[pts]
        for k in range(1, KMAX + 1):
            srcs = pts[ks == k]  # already in slot order
            nk = len(srcs)
            if nk == 0:
                continue
            gidx = (start[srcs][:, None] + np.arange(k)[None, :]).reshape(-1)
            omap[out_off[k] : out_off[k] + nk * k] = order3[gidx]
        outmaps.append(omap)

    key = (n1p, n2p, n3p, slabs)

    # shared weights
    W1a = np.zeros((258, 192), np.float32)
    W1a[:, :129] = W1[:258]
    W1b = np.ascontiguousarray(W1[258:770])
    W2a = np.ascontiguousarray(W2[:129])
    W2b = np.ascontiguousarray(W2[129:385])
    W3a = np.zeros((64, 64), np.float32)
    W3a[:, :34] = W3[:64]
    W3a_bf = W3a.astype(ml_dtypes.bfloat16)
    W3b = np.ascontiguousarray(W3[64:192])
    bn1 = np.stack([g1, be1], 1)
    bn2 = np.stack([g2, be2], 1)
    b3p = np.ascontiguousarray(b3.reshape(34, 1))

    featsTf = np.ascontiguousarray(feats.T)  # [258, 4096]
    s3Tf = skip3.T
    s2Tf = skip2.T
    s1Tf = skip1.T

    pl1 = _gplan(0, n1p)
    pl2 = _gplan(0, n2p)

    in_maps = []
    for c in range(NCORES):
        p1, p2 = P1[c], P2[c]
        k1 = len(p1)
        omap = outmaps[c]

        s3T = np.zeros((512, n1p), np.float32)
        s3T[:, :k1] = s3Tf[:, p1]
        s2T = np.zeros((256, n2p), np.float32)
        s2T[:, slot2[p2]] = s2Tf[:, p2]
        s1T = np.zeros((128, n3p), np.float32)
        valid = omap >= 0
        s1T[:, valid] = s1Tf[:, omap[valid]]

        g1i = np.full(n1p, 512, np.int64)
        g1i[:k1] = _perm_pm(idx1[p1] - C3 * c, 4)

        g2i = np.full(n2p, n1p, np.int64)
        g2i[slot2[p2]] = _perm_pm(pos1[idx2[p2]], T1)

        in_maps.append(
            {
                "featsT": np.ascontiguousarray(featsTf[:, C3 * c : C3 * (c + 1)]),
                "s3T": s3T,
                "s2T": s2T,
                "s1T": s1T,
                "gi1": _wrap_idx(g1i, pl1),
                "gi2": _wrap_idx(g2i, pl2),
                "W1a": W1a,
                "W1b": W1b,
                "W2a": W2a,
                "W2b": W2b,
                "W3a": W3a_bf,
                "W3b": W3b,
                "bn1": bn1,
                "bn2": bn2,
                "b3": b3p,
            }
        )

    return key, in_maps, outmaps


def _install_ntff_hook():
    """The image's antenv lacks axon_hooks; shim it so trace=True can capture
    NTFF profiles through the axon .so (same path trn_boot would register)."""
    import sys as _sys
    import types

    if "antenv.axon_hooks" in _sys.modules:
        return
    mod = types.ModuleType("antenv.axon_hooks")
    holder = {}
    mod.set_axon_ntff_profile_hook = lambda h: holder.__setitem__("h", h)
    mod.get_axon_ntff_profile_hook = lambda: holder.get("h")
    _sys.modules["antenv.axon_hooks"] = mod
    try:
        from trn_agent_boot.trn_boot import _ntff_profile_via_ctypes

        h = _ntff_profile_via_ctypes("/opt/axon/libaxon_pjrt.so")
        if h is not None:
            holder["h"] = h
    except Exception:
        pass


def kernel(_want_trace=False, **inputs):
    import os

    if _want_trace:
        _install_ntff_hook()
    key, in_maps, outmaps = prepare(**inputs)
    stop_after = os.environ.get("K_STOP_AFTER") or None
    no_prep = not os.environ.get("K_PREP")
    key2 = key + (stop_after, no_prep)
    nc = _get_program(key2, *key, stop_after, no_prep)

    res = bass_utils.run_bass_kernel_spmd(
        nc, in_maps, core_ids=list(range(NCORES)), trace=_want_trace
    )

    out = np.empty((N0, 34), np.float32)
    for c in range(NCORES):
        omap = outmaps[c]
        valid = omap >= 0
        out[omap[valid]] = np.asarray(res.results[c]["out"], np.float32).T[valid]

    if _want_trace:
        kernel._last_trace = res
    return out
